# revision 7
# baseline (speedup 1.0000x reference)
"""ATLoss (segment-max pooled multi-label loss) on 8 Trainium2 NeuronCores.

Problem shapes (hardcoded): logits [524288, 97] f32, labels [65536, 97] f32,
pos [65536, 2] int (contiguous segments of 8 rows each, tiling logits rows).

Sharding: core i takes segments [i*8192, (i+1)*8192) == logits rows
[i*65536, (i+1)*65536). Each core produces per-partition partial sums
[128, 2] = (sum loss1, sum loss2); the host combines the two scalar means.

Math per core (E_c = 8192 segments, C = 97, K = 8 rows/segment):
  nmask     = labels*NEG with col 0 zeroed   (1e30 at positive classes)
  m         = logits - nmask                  (mask positives away)
  S2[row]   = sum_c exp(m);  loss2 = sum_rows (ln S2 - logits[:, 0])
  smax      = segment max over K rows of logits
  e1        = smax + (nmask - NEG with col0=0)   (mask negatives away)
  S1[seg]   = sum_c exp(e1)
  loss1     = sum_segs (npos * ln S1 - 1e-30 * sum_c nmask*smax)
No max-subtraction needed before exp: logits ~ N(0,1) so exp is safe, and
exp(-1e30) underflows to exactly 0 which implements the mask.
"""

import numpy as np

E, C, K = 65536, 97, 8
N_ROWS = E * K
NCORES = 8
E_CORE = E // NCORES          # 8192 segments per core
R_CORE = E_CORE * K           # 65536 logits rows per core
P = 128                       # SBUF partitions
S_P = E_CORE // P             # 64 segments per partition
T = 8                         # segments per partition per tile
NTILES = S_P // T
NEG = 1e30


def build_nc():
    import concourse.bacc as bacc
    import concourse.mybir as mybir
    import concourse.tile as tile

    f32 = mybir.dt.float32
    Alu = mybir.AluOpType
    Act = mybir.ActivationFunctionType
    X = mybir.AxisListType.X
    XY = mybir.AxisListType.XY

    nc = bacc.Bacc()
    logits = nc.dram_tensor("logits", [R_CORE, C], f32, kind="ExternalInput")
    labels = nc.dram_tensor("labels", [E_CORE, C], f32, kind="ExternalInput")
    out = nc.dram_tensor("out", [P, 2], f32, kind="ExternalOutput")

    lg = logits[:].rearrange("(p r) c -> p r c", p=P)   # [128, 512, 97]
    lb = labels[:].rearrange("(p s) c -> p s c", p=P)   # [128, 64, 97]

    with tile.TileContext(nc) as tc:
        with (
            tc.tile_pool(name="resident", bufs=1) as resident,
            tc.tile_pool(name="big", bufs=2) as big,
            tc.tile_pool(name="scratch", bufs=1) as scratch,
            tc.tile_pool(name="med", bufs=2) as med,
            tc.tile_pool(name="small", bufs=2) as small,
        ):
            # Resident nmask = labels*NEG with col0 zeroed; npos = #positives.
            nmask = resident.tile([P, S_P, C], f32)
            nc.sync.dma_start(out=nmask, in_=lb)
            nc.vector.memset(nmask[:, :, 0], 0.0)
            npos = resident.tile([P, S_P], f32)
            nc.vector.tensor_reduce(out=npos, in_=nmask, axis=X, op=Alu.add)
            nc.vector.tensor_scalar_mul(nmask, nmask, NEG)

            acc1 = resident.tile([P, 1], f32)
            acc2 = resident.tile([P, 1], f32)
            nc.vector.memset(acc1, 0.0)
            nc.vector.memset(acc2, 0.0)

            for t in range(NTILES):
                # ---- load logits tile: T segments/partition ----
                L = big.tile([P, T, K, C], f32, tag="L")
                nc.sync.dma_start(
                    out=L, in_=lg[:, t * T * K:(t + 1) * T * K, :]
                )
                nm_t = nmask[:, t * T:(t + 1) * T, :]          # [P, T, C]

                # ---- loss2: m = logits - nmask (broadcast over K) ----
                m = big.tile([P, T, K, C], f32, tag="m")
                nm_b = nm_t.unsqueeze(2).broadcast_to((P, T, K, C))
                nc.vector.tensor_tensor(
                    out=m, in0=L, in1=nm_b, op=Alu.subtract
                )
                mf = m.rearrange("p t k c -> p (t k c)")
                nc.scalar.activation(out=mf, in_=mf, func=Act.Exp)
                S2 = med.tile([P, T * K], f32, tag="S2")
                nc.vector.tensor_reduce(out=S2, in_=m, axis=X, op=Alu.add)
                logS2 = med.tile([P, T * K], f32, tag="logS2")
                r1 = small.tile([P, 1], f32, tag="r1")
                nc.scalar.activation(
                    out=logS2, in_=S2, func=Act.Ln, accum_out=r1
                )
                r0 = small.tile([P, 1], f32, tag="r0")
                nc.vector.tensor_reduce(
                    out=r0, in_=L[:, :, :, 0], axis=XY, op=Alu.add
                )
                nc.vector.tensor_tensor(out=acc2, in0=acc2, in1=r1, op=Alu.add)
                nc.vector.tensor_tensor(
                    out=acc2, in0=acc2, in1=r0, op=Alu.subtract
                )

                # ---- segment max via pairwise max tree ----
                mx4 = scratch.tile([P, T, 4, C], f32, tag="mx4")
                nc.vector.tensor_tensor(
                    out=mx4, in0=L[:, :, 0:4, :], in1=L[:, :, 4:8, :],
                    op=Alu.max,
                )
                mx2 = scratch.tile([P, T, 2, C], f32, tag="mx2")
                nc.vector.tensor_tensor(
                    out=mx2, in0=mx4[:, :, 0:2, :], in1=mx4[:, :, 2:4, :],
                    op=Alu.max,
                )
                smax = med.tile([P, T, C], f32, tag="smax")
                nc.vector.tensor_tensor(
                    out=smax, in0=mx2[:, :, 0, :], in1=mx2[:, :, 1, :],
                    op=Alu.max,
                )

                # ---- loss1 ----
                mask1 = med.tile([P, T, C], f32, tag="mask1")
                nc.vector.tensor_scalar_sub(mask1, nm_t, NEG)
                nc.vector.memset(mask1[:, :, 0], 0.0)
                e1 = med.tile([P, T, C], f32, tag="e1")
                nc.vector.tensor_tensor(
                    out=e1, in0=smax, in1=mask1, op=Alu.add
                )
                e1f = e1.rearrange("p t c -> p (t c)")
                nc.scalar.activation(out=e1f, in_=e1f, func=Act.Exp)
                S1 = small.tile([P, T], f32, tag="S1")
                nc.vector.tensor_reduce(out=S1, in_=e1, axis=X, op=Alu.add)
                logS1 = small.tile([P, T], f32, tag="logS1")
                nc.scalar.activation(out=logS1, in_=S1, func=Act.Ln)
                tl = med.tile([P, T, C], f32, tag="tl")
                nc.vector.tensor_tensor(
                    out=tl, in0=nm_t, in1=smax, op=Alu.mult
                )
                tt = small.tile([P, T], f32, tag="tt")
                nc.vector.tensor_reduce(out=tt, in_=tl, axis=X, op=Alu.add)
                u = small.tile([P, T], f32, tag="u")
                nc.vector.tensor_tensor(
                    out=u, in0=npos[:, t * T:(t + 1) * T], in1=logS1,
                    op=Alu.mult,
                )
                v = small.tile([P, T], f32, tag="v")
                nc.vector.scalar_tensor_tensor(
                    out=v, in0=tt, scalar=-1e-30, in1=u,
                    op0=Alu.mult, op1=Alu.add,
                )
                r2 = small.tile([P, 1], f32, tag="r2")
                nc.vector.tensor_reduce(out=r2, in_=v, axis=X, op=Alu.add)
                nc.vector.tensor_tensor(out=acc1, in0=acc1, in1=r2, op=Alu.add)

            outsb = resident.tile([P, 2], f32)
            nc.vector.tensor_copy(outsb[:, 0:1], acc1)
            nc.vector.tensor_copy(outsb[:, 1:2], acc2)
            nc.sync.dma_start(out=out[:], in_=outsb)

    nc.finalize()
    return nc


def _numpy_fallback(logits, labels, pos):
    """Exact host computation for non-uniform (but contiguous) segments."""
    logits = np.asarray(logits, np.float64)
    labels = np.asarray(labels, np.float64).copy()
    pos = np.asarray(pos, np.int64)
    starts = pos[:, 0]
    lens = pos[:, 1] - pos[:, 0]
    seg_ids = np.repeat(np.arange(E), lens)[:N_ROWS]

    labels[:, 0] = 0.0
    p_mask = labels.copy()
    p_mask[:, 0] = 1.0

    e_logits = np.maximum.reduceat(logits, starts, axis=0)
    e1 = e_logits - (1.0 - p_mask) * NEG
    mx = e1.max(axis=1, keepdims=True)
    lse1 = np.log(np.exp(e1 - mx).sum(axis=1, keepdims=True)) + mx
    loss1 = ((lse1 - e1) * labels).sum(axis=1)

    m = logits - labels[seg_ids] * NEG
    mx2 = m.max(axis=1, keepdims=True)
    lse2 = np.log(np.exp(m - mx2).sum(axis=1)) + mx2[:, 0]
    loss2 = lse2 - m[:, 0]

    return np.float32(loss1.mean() + loss2.mean())


_NC_CACHE = {}


def kernel(logits, labels, pos):
    logits = np.ascontiguousarray(np.asarray(logits, dtype=np.float32))
    labels = np.ascontiguousarray(np.asarray(labels, dtype=np.float32))
    pos_np = np.asarray(pos)

    starts = pos_np[:, 0].astype(np.int64)
    ends = pos_np[:, 1].astype(np.int64)
    uniform = bool(
        starts[0] == 0
        and np.all(ends - starts == K)
        and np.all(starts == K * np.arange(E, dtype=np.int64))
    )
    if not uniform:
        return _numpy_fallback(logits, labels, pos_np)

    from concourse.bass_utils import run_bass_kernel_spmd

    if "nc" not in _NC_CACHE:
        _NC_CACHE["nc"] = build_nc()
    nc = _NC_CACHE["nc"]

    in_maps = [
        {
            "logits": logits[i * R_CORE:(i + 1) * R_CORE],
            "labels": labels[i * E_CORE:(i + 1) * E_CORE],
        }
        for i in range(NCORES)
    ]
    res = run_bass_kernel_spmd(nc, in_maps, list(range(NCORES)))
    parts = np.stack([r["out"] for r in res.results])  # [8, 128, 2]
    sums = parts.astype(np.float64).sum(axis=(0, 1))
    total = sums[0] / E + sums[1] / N_ROWS
    return np.float32(total)


# revision 8
# speedup vs baseline: 1.6493x; 1.6493x over previous
"""ATLoss (segment-max pooled multi-label loss) on 8 Trainium2 NeuronCores.

Problem shapes (hardcoded): logits [524288, 97] f32, labels [65536, 97] f32,
pos [65536, 2] int (contiguous segments of 8 rows each, tiling logits rows).

Sharding: core i takes segments [i*8192, (i+1)*8192) == logits rows
[i*65536, (i+1)*65536). Each core produces per-partition partial sums
[128, 2] = (sum loss1, sum loss2); the host combines the two scalar means.

V2: fp16 on-chip. Host casts logits/labels to fp16 (halves HBM traffic and
unlocks the DVE 2x_1P perf mode for tensor_tensor); all sums accumulate in
fp32. Mask constant NEGF = 32768 = 2^15: exactly representable in fp16,
exp(x - 32768) underflows to exactly 0, nmask*2^-15 recovers labels0
exactly, and no product overflows fp16 range.

Math per core (E_c = 8192 segments, C = 97, K = 8 rows/segment):
  nmask     = labels*NEGF with col 0 zeroed  (32768 at positive classes)
  m         = logits - nmask                 (mask positives away)
  S2[row]   = sum_c exp(m);  loss2 = sum_rows (ln S2 - logits[:, 0])
  smax      = segment max over K rows of logits (pairwise max tree)
  e1        = smax + (nmask - NEGF with col0=0)  (mask negatives away)
  S1[seg]   = sum_c exp(e1)
  loss1     = sum_segs (npos * ln S1 - sum_c (nmask*2^-15)*smax)
Row sums over C=97 are computed with an in-place pairwise add tree
(48+24+12+6+3 halvings + tail) so the DVE runs in 2x mode instead of the
1x-only tensor_reduce.
"""

import numpy as np

E, C, K = 65536, 97, 8
N_ROWS = E * K
NCORES = 8
E_CORE = E // NCORES          # 8192 segments per core
R_CORE = E_CORE * K           # 65536 logits rows per core
P = 128                       # SBUF partitions
S_P = E_CORE // P             # 64 segments per partition
T = 16                        # segments per partition per tile
NTILES = S_P // T
NEGF = 32768.0                # 2^15
INV_NEGF = 1.0 / 32768.0      # 2^-15, exact


def build_nc():
    import concourse.bacc as bacc
    import concourse.mybir as mybir
    import concourse.tile as tile

    f32 = mybir.dt.float32
    f16 = mybir.dt.float16
    Alu = mybir.AluOpType
    Act = mybir.ActivationFunctionType
    X = mybir.AxisListType.X
    XY = mybir.AxisListType.XY

    nc = bacc.Bacc()
    logits = nc.dram_tensor("logits", [R_CORE, C], f16, kind="ExternalInput")
    labels = nc.dram_tensor("labels", [E_CORE, C], f16, kind="ExternalInput")
    out = nc.dram_tensor("out", [P, 2], f32, kind="ExternalOutput")

    lg = logits[:].rearrange("(p r) c -> p r c", p=P)   # [128, 512, 97]
    lb = labels[:].rearrange("(p s) c -> p s c", p=P)   # [128, 64, 97]

    R = T * K  # rows per partition per tile

    with tile.TileContext(nc) as tc:
        with (
            tc.tile_pool(name="resident", bufs=1) as resident,
            tc.tile_pool(name="big", bufs=2) as big,
            tc.tile_pool(name="scratch", bufs=1) as scratch,
            tc.tile_pool(name="med", bufs=2) as med,
            tc.tile_pool(name="small", bufs=2) as small,
        ):
            # Resident nmask = labels*NEGF with col0 zeroed; npos.
            nmask = resident.tile([P, S_P, C], f16)
            nc.sync.dma_start(out=nmask, in_=lb)
            nc.vector.memset(nmask[:, :, 0], 0.0)
            npos = resident.tile([P, S_P], f32)
            nc.vector.tensor_reduce(out=npos, in_=nmask, axis=X, op=Alu.add)
            nc.vector.tensor_scalar_mul(nmask, nmask, NEGF)

            acc1 = resident.tile([P, 1], f32)
            acc2 = resident.tile([P, 1], f32)
            nc.vector.memset(acc1, 0.0)
            nc.vector.memset(acc2, 0.0)

            for t in range(NTILES):
                # ---- load logits tile: T segments/partition ----
                L = big.tile([P, T, K, C], f16, tag="L")
                nc.sync.dma_start(
                    out=L, in_=lg[:, t * R:(t + 1) * R, :]
                )
                nm_t = nmask[:, t * T:(t + 1) * T, :]          # [P, T, C]

                # ---- loss2: m = logits - nmask (broadcast over K) ----
                m = big.tile([P, T, K, C], f16, tag="m")
                nm_b = nm_t.unsqueeze(2).broadcast_to((P, T, K, C))
                nc.vector.tensor_tensor(
                    out=m, in0=L, in1=nm_b, op=Alu.subtract
                )
                mf = m.rearrange("p t k c -> p (t k c)")
                nc.scalar.activation(out=mf, in_=mf, func=Act.Exp)

                # sum over col 0 of raw logits (loss2's -x0 term)
                r0 = small.tile([P, 1], f32, tag="r0")
                nc.vector.tensor_reduce(
                    out=r0, in_=L[:, :, :, 0], axis=XY, op=Alu.add
                )

                # ---- per-row sum over C: in-place pairwise add tree ----
                z = m.rearrange("p t k c -> p (t k) c")       # [P, R, C]
                for w in (48, 24, 12, 6, 3):
                    nc.vector.tensor_tensor(
                        out=z[:, :, 0:w], in0=z[:, :, 0:w],
                        in1=z[:, :, w:2 * w], op=Alu.add,
                    )
                nc.vector.tensor_tensor(
                    out=z[:, :, 0:1], in0=z[:, :, 0:1], in1=z[:, :, 1:2],
                    op=Alu.add,
                )
                nc.vector.tensor_tensor(
                    out=z[:, :, 0:1], in0=z[:, :, 0:1], in1=z[:, :, 2:3],
                    op=Alu.add,
                )
                S2 = med.tile([P, R], f32, tag="S2")
                nc.vector.tensor_tensor(
                    out=S2, in0=z[:, :, 0], in1=z[:, :, 96], op=Alu.add
                )
                logS2 = med.tile([P, R], f32, tag="logS2")
                r1 = small.tile([P, 1], f32, tag="r1")
                nc.scalar.activation(
                    out=logS2, in_=S2, func=Act.Ln, accum_out=r1
                )
                nc.vector.tensor_tensor(out=acc2, in0=acc2, in1=r1, op=Alu.add)
                nc.vector.tensor_tensor(
                    out=acc2, in0=acc2, in1=r0, op=Alu.subtract
                )

                # ---- segment max via pairwise max tree ----
                mx4 = scratch.tile([P, T, 4, C], f16, tag="mx4")
                nc.vector.tensor_tensor(
                    out=mx4, in0=L[:, :, 0:4, :], in1=L[:, :, 4:8, :],
                    op=Alu.max,
                )
                mx2 = scratch.tile([P, T, 2, C], f16, tag="mx2")
                nc.vector.tensor_tensor(
                    out=mx2, in0=mx4[:, :, 0:2, :], in1=mx4[:, :, 2:4, :],
                    op=Alu.max,
                )
                smax = med.tile([P, T, C], f16, tag="smax")
                nc.vector.tensor_tensor(
                    out=smax, in0=mx2[:, :, 0, :], in1=mx2[:, :, 1, :],
                    op=Alu.max,
                )

                # ---- loss1 ----
                mask1 = med.tile([P, T, C], f16, tag="mask1")
                nc.vector.tensor_scalar_sub(mask1, nm_t, NEGF)
                nc.vector.memset(mask1[:, :, 0], 0.0)
                e1 = med.tile([P, T, C], f16, tag="e1")
                nc.vector.tensor_tensor(
                    out=e1, in0=smax, in1=mask1, op=Alu.add
                )
                e1f = e1.rearrange("p t c -> p (t c)")
                nc.scalar.activation(out=e1f, in_=e1f, func=Act.Exp)
                S1 = small.tile([P, T], f32, tag="S1")
                nc.vector.tensor_reduce(out=S1, in_=e1, axis=X, op=Alu.add)
                logS1 = small.tile([P, T], f32, tag="logS1")
                nc.scalar.activation(out=logS1, in_=S1, func=Act.Ln)
                # t-term: sum_c labels0*smax == sum_c (nmask*2^-15)*smax
                tl = med.tile([P, T, C], f16, tag="tl")
                nc.vector.scalar_tensor_tensor(
                    out=tl, in0=nm_t, scalar=INV_NEGF, in1=smax,
                    op0=Alu.mult, op1=Alu.mult,
                )
                tt = small.tile([P, T], f32, tag="tt")
                nc.vector.tensor_reduce(out=tt, in_=tl, axis=X, op=Alu.add)
                u = small.tile([P, T], f32, tag="u")
                nc.vector.tensor_tensor(
                    out=u, in0=npos[:, t * T:(t + 1) * T], in1=logS1,
                    op=Alu.mult,
                )
                v = small.tile([P, T], f32, tag="v")
                nc.vector.tensor_tensor(out=v, in0=u, in1=tt, op=Alu.subtract)
                r2 = small.tile([P, 1], f32, tag="r2")
                nc.vector.tensor_reduce(out=r2, in_=v, axis=X, op=Alu.add)
                nc.vector.tensor_tensor(out=acc1, in0=acc1, in1=r2, op=Alu.add)

            outsb = resident.tile([P, 2], f32)
            nc.vector.tensor_copy(outsb[:, 0:1], acc1)
            nc.vector.tensor_copy(outsb[:, 1:2], acc2)
            nc.sync.dma_start(out=out[:], in_=outsb)

    nc.finalize()
    return nc


def _numpy_fallback(logits, labels, pos):
    """Exact host computation for non-uniform (but contiguous) segments."""
    logits = np.asarray(logits, np.float64)
    labels = np.asarray(labels, np.float64).copy()
    pos = np.asarray(pos, np.int64)
    starts = pos[:, 0]
    lens = pos[:, 1] - pos[:, 0]
    seg_ids = np.repeat(np.arange(E), lens)[:N_ROWS]

    labels[:, 0] = 0.0
    p_mask = labels.copy()
    p_mask[:, 0] = 1.0
    NEG = 1e30

    e_logits = np.maximum.reduceat(logits, starts, axis=0)
    e1 = e_logits - (1.0 - p_mask) * NEG
    mx = e1.max(axis=1, keepdims=True)
    lse1 = np.log(np.exp(e1 - mx).sum(axis=1, keepdims=True)) + mx
    loss1 = ((lse1 - e1) * labels).sum(axis=1)

    m = logits - labels[seg_ids] * NEG
    mx2 = m.max(axis=1, keepdims=True)
    lse2 = np.log(np.exp(m - mx2).sum(axis=1)) + mx2[:, 0]
    loss2 = lse2 - m[:, 0]

    return np.float32(loss1.mean() + loss2.mean())


_NC_CACHE = {}


def kernel(logits, labels, pos):
    pos_np = np.asarray(pos)
    starts = pos_np[:, 0].astype(np.int64)
    ends = pos_np[:, 1].astype(np.int64)
    uniform = bool(
        starts[0] == 0
        and np.all(ends - starts == K)
        and np.all(starts == K * np.arange(E, dtype=np.int64))
    )
    if not uniform:
        return _numpy_fallback(logits, labels, pos_np)

    logits16 = np.ascontiguousarray(
        np.asarray(logits, dtype=np.float32).astype(np.float16)
    )
    labels16 = np.ascontiguousarray(
        np.asarray(labels, dtype=np.float32).astype(np.float16)
    )

    from concourse.bass_utils import run_bass_kernel_spmd

    if "nc" not in _NC_CACHE:
        _NC_CACHE["nc"] = build_nc()
    nc = _NC_CACHE["nc"]

    in_maps = [
        {
            "logits": logits16[i * R_CORE:(i + 1) * R_CORE],
            "labels": labels16[i * E_CORE:(i + 1) * E_CORE],
        }
        for i in range(NCORES)
    ]
    res = run_bass_kernel_spmd(nc, in_maps, list(range(NCORES)))
    parts = np.stack([r["out"] for r in res.results])  # [8, 128, 2]
    sums = parts.astype(np.float64).sum(axis=(0, 1))
    total = sums[0] / E + sums[1] / N_ROWS
    return np.float32(total)
